# revision 17
# baseline (speedup 1.0000x reference)
"""Last-query sparse attention on 8 TRN2 NeuronCores.

Reference computation (per sample b):
    prev  = x[b, :-1, :]                 # [T-1, D]
    final = x[b, -1, :]                  # [D]
    s     = prev @ final                 # [T-1]
    w     = softmax(s)
    att   = w @ prev                     # [D]
    out   = concat(final, att)           # [2D]

Sharding: batch (B=64) split 8 ways -> 8 samples per core, no collectives.

v5 design (trace-driven):
- DMA: 16-block (2MB) SWDGE cast chunks (8KB write packets run the SDMA
  engines at their ~26GB/s limit; bigger packets measure ~20% slower).
  Samples 0-5 front-loaded (xbpool bufs=6); 6/7 triggers drop in behind
  early all-reduces. Sample 7 loads as 16/8/4/4 pieces so the
  last-arriving piece needs minimal work.
- Pass 1 on DVE: chunk-split fp16 mul+l1 (pipeline granularity at the
  data-arrival pinch), merged l2/l3/l4 + 1x segmented reduce to fp16 S.
- All 8 Fh broadcasts built up front (ACT FIFO otherwise parks them
  behind matmul-dependent stage copies).
- Softmax samples 0-6: DVE row-max -> gpsimd partition_all_reduce ->
  ACT negate -> ACT exp. Sample 7 pieces use a deterministic
  PE-transpose max chain (rm -> PE transpose -> DVE reduce_max -> PE
  ones-broadcast -> ACT negate) because tail AR latency is 1-5.6us.
- PE keep-warm: tiny dummy matmuls anchored on S of samples 5-7 only
  (S has a 3-deep pool so the PE reader can't stall future DVE work;
  anchoring on 2-deep scr tiles serialized the whole pipeline).
- Pass 2: 32 PE matmuls/sample into a [1,512] PSUM row + ones-matmul Z.
  Sample 7: per-piece banks with zero-padded 16-wide Z, single end
  combine (3 STT adds over [0:272]), Z via one ACT accum_out.
"""

import sys

sys.path.insert(0, "/opt/trn_rl_repo")

from contextlib import ExitStack

import numpy as np

import concourse.tile as tile
import concourse.bass_isa as bass_isa
from concourse import bacc, mybir
from concourse.bass_utils import run_bass_kernel_spmd

N_CORES = 8
B = 64
T = 4096
D = 256
BPC = B // N_CORES  # samples per core
P = 128
NBLK = T // P  # 32 blocks; t = p*NBLK + i
CB = 16  # blocks per chunk
F32 = mybir.dt.float32
FP16 = mybir.dt.float16
AX = mybir.AxisListType
ALU = mybir.AluOpType

_NC_CACHE = None

# sample-7 flash pieces (block ranges)
S7_PIECES = [(0, 16), (16, 24), (24, 28), (28, 32)]


def _build():
    AF = mybir.ActivationFunctionType
    nc = bacc.Bacc(
        trn_type="TRN2",
        target_bir_lowering=False,
        debug=False,
        num_devices=N_CORES,
    )
    x_ext = nc.declare_dram_parameter("x", [BPC, T, D], F32, isOutput=False)
    out_ext = nc.declare_dram_parameter("out", [BPC, 2 * D], F32, isOutput=True)
    xap = x_ext.ap()
    oap = out_ext.ap()

    with ExitStack() as ctx:
        tc = ctx.enter_context(tile.TileContext(nc))
        xbpool = ctx.enter_context(tc.tile_pool(name="xbp", bufs=6))
        fpool = ctx.enter_context(tc.tile_pool(name="fp", bufs=1))
        scrpool = ctx.enter_context(tc.tile_pool(name="scr", bufs=2))
        spool = ctx.enter_context(tc.tile_pool(name="sp", bufs=3))
        stat = ctx.enter_context(tc.tile_pool(name="stat", bufs=3))
        cpool = ctx.enter_context(tc.tile_pool(name="const", bufs=1))
        pspool = ctx.enter_context(tc.tile_pool(name="ps", bufs=4, space="PSUM"))
        psf_pool = ctx.enter_context(tc.tile_pool(name="psf", bufs=1, space="PSUM"))

        xr = [xap[b].rearrange("(p i) d -> p i d", p=P) for b in range(BPC)]

        # --- earliest DMAs ------------------------------------------------
        # final-row gather on the HWDGE/sync path first: it feeds fh_bcast
        F_all = cpool.tile([1, BPC, D], F32, tag="fall")
        nc.sync.dma_start(F_all[0:1, :, :], xap[:, T - 1, :].unsqueeze(0))

        Xh = [xbpool.tile([P, NBLK, D], FP16, tag="xh", name="xh") for _ in range(BPC)]

        def trig(b, lo, hi):
            nc.gpsimd.dma_start(Xh[b][:, lo:hi, :], xr[b][:, lo:hi, :])

        # sample 0 ramps with two 8-block pieces; 1-5 as 16-block chunks
        trig(0, 0, 8)
        trig(0, 8, 16)
        trig(0, 16, 32)
        for b in range(1, 6):
            trig(b, 0, CB)
            trig(b, CB, NBLK)

        # final half of the output: straight HBM->HBM copy (sync engine)
        nc.sync.dma_start(oap[:, 0:D], xap[:, T - 1, :])

        # --- constants ----------------------------------------------------
        ones16 = cpool.tile([P, 1], FP16, tag="ones16")
        nc.vector.memset(ones16[:], 1.0)
        onesf = cpool.tile([1, P], F32, tag="onesf")
        nc.vector.memset(onesf[:], 1.0)
        maskbias = cpool.tile([P, 1], FP16, tag="mb")
        nc.vector.memset(maskbias[:], 0.0)
        neg6e4 = cpool.tile([1, 1], FP16, tag="neg6e4")
        nc.vector.memset(neg6e4[:], -60000.0)
        # DMA can write partition 127 where engine APs cannot
        nc.sync.dma_start(maskbias[P - 1 : P, 0:1], neg6e4[0:1, 0:1])
        # identity for PE-transpose (tail max chain)
        ident = cpool.tile([P, P], F32, tag="ident")
        nc.gpsimd.memset(ident[:], 0.0)
        nc.gpsimd.affine_select(
            out=ident[:],
            in_=ident[:],
            compare_op=ALU.not_equal,
            fill=1.0,
            base=0,
            pattern=[[-1, P]],
            channel_multiplier=1,
        )

        # partition-0 staging: [0:256] unnormalized att, [256:288] Z partials
        stage = cpool.tile([1, BPC, 288], F32, tag="stage")
        att_n = cpool.tile([1, BPC, D], F32, tag="attn")

        # all 8 Fh broadcasts up front: PE ones-matmul + ACT cast copy
        Fh = []
        for b in range(BPC):
            ps = psf_pool.tile([P, D], F32, tag="psf", name=f"psfb{b}")
            nc.tensor.matmul(ps[:], lhsT=onesf[:], rhs=F_all[0:1, b, :], start=True, stop=True)
            fh = fpool.tile([P, D], FP16, tag=f"fh{b}", name=f"fh{b}")
            nc.scalar.activation(fh[:], ps[:], AF.Copy)
            Fh.append(fh)

        def keep_warm(anchor_ap, idx):
            # tiny matmul reading S (3-deep pool -> no DVE coupling):
            # keeps the PE HAM window from ever going fully idle near the tail
            dps = psf_pool.tile([1, 64], F32, tag="dum", name=f"dum{idx}")
            w = anchor_ap.free_size()
            nc.tensor.matmul(
                dps[0:1, 0:w], lhsT=ones16[:], rhs=anchor_ap, start=True, stop=True
            )

        rz = stat.tile([1, BPC], F32, tag="rz", bufs=1)

        def epilogue(b):
            # Z(b) via ACT accum_out (keeps DVE to just the reciprocal);
            # ACT then does the normalize copy
            zb = stat.tile([1, 1], F32, tag="zb", name="zb")
            zscr = stat.tile([1, NBLK], F32, tag="zscr", name="zscr")
            nc.scalar.activation(
                zscr[0:1, 0:NBLK], stage[0:1, b, D : D + NBLK], AF.Copy,
                accum_out=zb[:],
            )
            nc.vector.reciprocal(rz[0:1, b : b + 1], zb[:])
            nc.scalar.activation(
                att_n[0:1, b, :], stage[0:1, b, 0:D], AF.Copy,
                scale=rz[0:1, b : b + 1],
            )

        def mul_l1(b, lo, hi, prod, l1):
            w = hi - lo
            nc.vector.tensor_mul(
                prod[:, lo:hi, :],
                Xh[b][:, lo:hi, :],
                Fh[b][:].unsqueeze(1).broadcast_to((P, w, D)),
            )
            nc.vector.tensor_add(
                l1[:, lo:hi, :], prod[:, lo:hi, 0 : D // 2], prod[:, lo:hi, D // 2 : D]
            )

        def tree_tail(lo, hi, l1, l2, l3, l4, S, mid_eng=None):
            # mid_eng=gpsimd offloads the l2/l3 levels off the saturated DVE
            # (slower per-op but fully parallel; DVE keeps mul/l1/l4/reduce)
            eng = mid_eng if mid_eng is not None else nc.vector
            eng.tensor_add(
                l2[:, lo:hi, :], l1[:, lo:hi, 0 : D // 4], l1[:, lo:hi, D // 4 : D // 2]
            )
            eng.tensor_add(
                l3[:, lo:hi, :], l2[:, lo:hi, 0 : D // 8], l2[:, lo:hi, D // 8 : D // 4]
            )
            nc.vector.tensor_add(
                l4[:, lo:hi, :], l3[:, lo:hi, 0 : D // 16], l3[:, lo:hi, D // 16 : D // 8]
            )
            with nc.allow_low_precision(reason="scores tree is already fp16"):
                nc.vector.reduce_sum(S[:, lo:hi], l4[:, lo:hi, :], axis=AX.X)

        def pass2(Pw, bank, b, lo, hi, zw):
            for i in range(lo, hi):
                nc.tensor.matmul(
                    bank[0:1, 0:D],
                    lhsT=Pw[:, i - lo : i - lo + 1],
                    rhs=Xh[b][:, i, :],
                    start=(i == lo),
                    stop=(i == hi - 1),
                )
            nc.tensor.matmul(
                bank[0:1, D : D + zw], lhsT=ones16[:], rhs=Pw[:, 0:zw],
                start=True, stop=True,
            )

        for b in range(BPC):
            S = spool.tile([P, NBLK], FP16, tag="s")
            prod = scrpool.tile([P, NBLK, D], FP16, tag="prod", name="prod")
            l1 = scrpool.tile([P, NBLK, D // 2], FP16, tag="l1", name="l1")
            l2 = scrpool.tile([P, NBLK, D // 4], FP16, tag="l2", name="l2")
            l3 = scrpool.tile([P, NBLK, D // 8], FP16, tag="l3", name="l3")
            l4 = scrpool.tile([P, NBLK, D // 16], FP16, tag="l4", name="l4")

            if b < BPC - 1:
                if b == 0:
                    mul_l1(b, 0, 8, prod, l1)
                    mul_l1(b, 8, 16, prod, l1)
                    mul_l1(b, 16, 32, prod, l1)
                else:
                    mul_l1(b, 0, CB, prod, l1)
                    mul_l1(b, CB, NBLK, prod, l1)
                tree_tail(0, NBLK, l1, l2, l3, l4, S, mid_eng=nc.gpsimd)

                # mask self-score (t=4095 -> p=127, i=31), row max
                nc.vector.tensor_add(
                    S[:, NBLK - 1 : NBLK], S[:, NBLK - 1 : NBLK], maskbias[:]
                )
                if b >= 5:
                    keep_warm(S[:, 0:4], f"{b}w")
                rm = stat.tile([P, 1], F32, tag="rm")
                nc.vector.reduce_max(rm[:], S[:], axis=AX.X)
                # cross-partition max on gpsimd (idle but for DMA triggers)
                gmax = stat.tile([P, 1], F32, tag="gm")
                nc.gpsimd.partition_all_reduce(
                    gmax[:], rm[:], channels=P, reduce_op=bass_isa.ReduceOp.max
                )
                # late DMA triggers behind early all-reduces so their
                # buffer-release waits never block a needed gpsimd op
                if b == 1:
                    trig(6, 0, CB)
                    trig(6, CB, NBLK)
                elif b == 2:
                    for lo, hi in S7_PIECES:
                        trig(7, lo, hi)
                negmax = stat.tile([P, 1], F32, tag="nm")
                nc.scalar.activation(negmax[:], gmax[:], AF.Copy, scale=-1.0)

                Pw = spool.tile([P, NBLK], FP16, tag="pw")
                nc.scalar.activation(Pw[:], S[:], AF.Exp, bias=negmax[:], scale=1.0)

                bank = pspool.tile([1, 512], F32, tag="attps", name="attps")
                pass2(Pw, bank, b, 0, NBLK, NBLK)
                nc.scalar.activation(stage[0:1, b, 0:288], bank[0:1, 0:288], AF.Copy)
            else:
                # ---- 4-piece flash tail: PE-transpose max chain ----------
                gcs = None  # cumulative max scalar [1,1]
                gs_list = []
                banks = []
                sbs = []
                for k, (lo, hi) in enumerate(S7_PIECES):
                    w = hi - lo
                    if k == 0:
                        mul_l1(b, 0, CB, prod, l1)
                    else:
                        mul_l1(b, lo, hi, prod, l1)
                    tree_tail(
                        lo, hi, l1, l2, l3, l4, S,
                        mid_eng=nc.gpsimd if k == 0 else None,
                    )
                    if hi == NBLK:
                        # mask self-score (t=4095 -> p=127, i=31)
                        nc.vector.tensor_add(
                            S[:, NBLK - 1 : NBLK], S[:, NBLK - 1 : NBLK], maskbias[:]
                        )
                    keep_warm(S[:, lo : lo + 4], f"7p{k}")
                    rmk = stat.tile([P, 1], F32, tag=f"rm7{k}", name=f"rm7{k}")
                    nc.vector.reduce_max(rmk[:], S[:, lo:hi], axis=AX.X)
                    # rm -> [1,128] psum -> scalar max on partition 0
                    pmax = psf_pool.tile([1, P], F32, tag="pmax", name=f"pmax{k}")
                    nc.tensor.transpose(pmax[:], rmk[:], ident[:])
                    gls = stat.tile([1, 1], F32, tag=f"gl7{k}", name=f"gl7{k}")
                    nc.vector.reduce_max(gls[:], pmax[0:1, :], axis=AX.X)
                    if k > 0:
                        gnew = stat.tile([1, 1], F32, tag=f"gc7{k}", name=f"gc7{k}")
                        nc.vector.tensor_max(gnew[:], gls[:], gcs[:])
                        gcs = gnew
                    else:
                        gcs = gls
                    gs_list.append(gcs)
                    # broadcast cumulative max to all partitions, negate
                    pbc = psf_pool.tile([P, 1], F32, tag="pbc", name=f"pbc{k}")
                    nc.tensor.matmul(pbc[:], lhsT=onesf[:], rhs=gcs[:], start=True, stop=True)
                    negk = stat.tile([P, 1], F32, tag=f"ng7{k}", name=f"ng7{k}")
                    nc.scalar.activation(negk[:], pbc[:], AF.Copy, scale=-1.0)
                    Pwk = spool.tile([P, CB], FP16, tag=f"pw7{k}", name=f"pw7{k}")
                    if w < CB:
                        nc.vector.memset(Pwk[:, w:CB], 0.0)
                    nc.scalar.activation(
                        Pwk[:, 0:w], S[:, lo:hi], AF.Exp, bias=negk[:], scale=1.0
                    )
                    bk = pspool.tile([1, 512], F32, tag="attps", name=f"att7{k}")
                    pass2(Pwk[:, 0:CB], bk, b, lo, hi, CB)
                    banks.append(bk)
                    if k < len(S7_PIECES) - 1:
                        sbk = stat.tile([1, D + CB], F32, tag=f"sb7{k}", name=f"sb7{k}")
                        nc.scalar.activation(sbk[:], bk[0:1, 0 : D + CB], AF.Copy)
                        sbs.append(sbk)
                # alphas a_k = exp(gm_k - gmF); final piece used the true max
                negF = stat.tile([1, 1], F32, tag="negF", name="negF")
                nc.scalar.activation(negF[:], gcs[:], AF.Copy, scale=-1.0)
                alphas = []
                for k in range(3):
                    ak = stat.tile([1, 1], F32, tag=f"al7{k}", name=f"al7{k}")
                    nc.scalar.activation(
                        ak[:], gs_list[k][:], AF.Exp, bias=negF[0:1, 0:1], scale=1.0
                    )
                    alphas.append(ak)
                # combine att+Z in one strip: stage7 = sum a_k*sb_k + bank3
                u1 = stat.tile([1, D + CB], F32, tag="u1", name="u1")
                nc.vector.scalar_tensor_tensor(
                    out=u1[:], in0=sbs[2][:], scalar=alphas[2][0:1, 0:1],
                    in1=banks[3][0:1, 0 : D + CB], op0=ALU.mult, op1=ALU.add,
                )
                u2 = stat.tile([1, D + CB], F32, tag="u2", name="u2")
                nc.vector.scalar_tensor_tensor(
                    out=u2[:], in0=sbs[1][:], scalar=alphas[1][0:1, 0:1],
                    in1=u1[:], op0=ALU.mult, op1=ALU.add,
                )
                nc.vector.scalar_tensor_tensor(
                    out=stage[0:1, b, 0 : D + CB], in0=sbs[0][:],
                    scalar=alphas[0][0:1, 0:1],
                    in1=u2[:], op0=ALU.mult, op1=ALU.add,
                )
                # Z, reciprocal, normalize
                zb7 = stat.tile([1, 1], F32, tag="zb7", name="zb7")
                zscr7 = stat.tile([1, CB], F32, tag="z7scr", name="z7scr")
                nc.scalar.activation(
                    zscr7[:], stage[0:1, b, D : D + CB], AF.Copy, accum_out=zb7[:]
                )
                nc.vector.reciprocal(rz[0:1, b : b + 1], zb7[:])
                nc.scalar.activation(
                    att_n[0:1, b, :], stage[0:1, b, 0:D], AF.Copy,
                    scale=rz[0:1, b : b + 1],
                )
            if 0 < b < BPC - 1:
                epilogue(b - 1)

        epilogue(BPC - 2)
        # rows 0..6 can fly as soon as their normalize copies land;
        # only sample 7's 1KB row remains on the tail
        nc.sync.dma_start(
            oap[0 : BPC - 1, D : 2 * D].unsqueeze(0), att_n[0:1, 0 : BPC - 1, :]
        )
        nc.sync.dma_start(
            oap[BPC - 1 : BPC, D : 2 * D].unsqueeze(0), att_n[0:1, BPC - 1, :].unsqueeze(1)
        )

    nc.compile()
    return nc


def _run(x, trace=False):
    global _NC_CACHE
    x = np.ascontiguousarray(np.asarray(x, dtype=np.float32))
    assert x.shape == (B, T, D), x.shape
    if _NC_CACHE is None:
        _NC_CACHE = _build()
    in_maps = [{"x": x[c * BPC : (c + 1) * BPC]} for c in range(N_CORES)]
    res = run_bass_kernel_spmd(
        _NC_CACHE, in_maps, core_ids=list(range(N_CORES)), trace=trace
    )
    out = np.concatenate([res.results[c]["out"] for c in range(N_CORES)], axis=0)
    return out.astype(np.float32), res


def kernel(x):
    out, _ = _run(x, trace=False)
    return out


# revision 22
# speedup vs baseline: 1.6817x; 1.6817x over previous
"""Last-query sparse attention on 8 TRN2 NeuronCores.

Reference computation (per sample b):
    prev  = x[b, :-1, :]                 # [T-1, D]
    final = x[b, -1, :]                  # [D]
    s     = prev @ final                 # [T-1]
    w     = softmax(s)
    att   = w @ prev                     # [D]
    out   = concat(final, att)           # [2D]

Sharding: batch (B=64) split 8 ways -> 8 samples per core, no collectives.

v7 design (trace-driven):
- DMA: 16-block (2MB) SWDGE cast chunks (8KB write packets run the SDMA
  engines at their ~26GB/s limit; bigger packets measure ~20% slower).
  Samples 0-5 front-loaded (xbpool bufs=6); sample 0 ramps 4/4/8/16.
  Sample 7 loads as 16/8/4/4 pieces so the last-arriving data needs
  minimal work.
- Pass 1 on DVE (the saturated engine, ~82us): chunk-split fp16 mul+l1,
  merged l2/l3/l4 + 1x segmented reduce to fp16 S. gpsimd tensor ops
  measured 2.4x WORSE overall -- do not offload tree levels there.
- All 8 Fh broadcasts built up front (ACT FIFO otherwise parks them
  behind matmul-dependent stage copies).
- Softmax samples 0-5: DVE row-max -> gpsimd partition_all_reduce ->
  ACT negate -> ACT exp (AR is off the critical path there). Samples
  6/7 use a deterministic PE-transpose max chain (rm -> PE transpose ->
  DVE reduce_max -> PE ones-broadcast -> ACT negate): tail ARs measured
  1-5.6us and their variance poisons the in-order PE/ACT queues.
- PE keep-warm: tiny dummy matmuls anchored on S of samples 5-7 only
  (S has a 3-deep pool so the PE reader can't stall future DVE work;
  anchoring on 2-deep scr tiles serialized the whole pipeline).
- Pass 2: 32 PE matmuls/sample into a [1,512] PSUM row + ones-matmul Z.
- Sample 7: 3 processing pieces (16/8/8; the last piece's muls split
  4+4 for data pipelining) with cumulative max, per-piece banks with
  zero-padded 16-wide Z, single end combine (2 STT adds over [0:272]),
  Z via one ACT accum_out.
"""

import sys

sys.path.insert(0, "/opt/trn_rl_repo")

from contextlib import ExitStack

import numpy as np

import concourse.tile as tile
import concourse.bass_isa as bass_isa
from concourse import bacc, mybir
from concourse.bass_utils import run_bass_kernel_spmd

N_CORES = 8
B = 64
T = 4096
D = 256
BPC = B // N_CORES  # samples per core
P = 128
NBLK = T // P  # 32 blocks; t = p*NBLK + i
CB = 16  # blocks per chunk
F32 = mybir.dt.float32
FP16 = mybir.dt.float16
AX = mybir.AxisListType
ALU = mybir.AluOpType

_NC_CACHE = None


def _build():
    AF = mybir.ActivationFunctionType
    nc = bacc.Bacc(
        trn_type="TRN2",
        target_bir_lowering=False,
        debug=False,
        num_devices=N_CORES,
    )
    x_ext = nc.declare_dram_parameter("x", [BPC, T, D], F32, isOutput=False)
    out_ext = nc.declare_dram_parameter("out", [BPC, 2 * D], F32, isOutput=True)
    xap = x_ext.ap()
    oap = out_ext.ap()

    with ExitStack() as ctx:
        tc = ctx.enter_context(tile.TileContext(nc))
        xbpool = ctx.enter_context(tc.tile_pool(name="xbp", bufs=6))
        fpool = ctx.enter_context(tc.tile_pool(name="fp", bufs=1))
        scrpool = ctx.enter_context(tc.tile_pool(name="scr", bufs=2))
        spool = ctx.enter_context(tc.tile_pool(name="sp", bufs=3))
        stat = ctx.enter_context(tc.tile_pool(name="stat", bufs=3))
        cpool = ctx.enter_context(tc.tile_pool(name="const", bufs=1))
        pspool = ctx.enter_context(tc.tile_pool(name="ps", bufs=4, space="PSUM"))
        psf_pool = ctx.enter_context(tc.tile_pool(name="psf", bufs=1, space="PSUM"))

        xr = [xap[b].rearrange("(p i) d -> p i d", p=P) for b in range(BPC)]

        # --- earliest DMAs ------------------------------------------------
        # final-row gather on the HWDGE/sync path first: it feeds fh_bcast
        F_all = cpool.tile([1, BPC, D], F32, tag="fall")
        nc.sync.dma_start(F_all[0:1, :, :], xap[:, T - 1, :].unsqueeze(0))

        Xh = [xbpool.tile([P, NBLK, D], FP16, tag="xh", name="xh") for _ in range(BPC)]

        def trig(b, lo, hi):
            nc.gpsimd.dma_start(Xh[b][:, lo:hi, :], xr[b][:, lo:hi, :])

        # sample 0 ramps 4/4/8/16; 1-5 as 16-block chunks
        trig(0, 0, 4)
        trig(0, 4, 8)
        trig(0, 8, 16)
        trig(0, 16, 32)
        for b in range(1, 6):
            trig(b, 0, CB)
            trig(b, CB, NBLK)

        # final half of the output: straight HBM->HBM copy (sync engine)
        nc.sync.dma_start(oap[:, 0:D], xap[:, T - 1, :])

        # --- constants ----------------------------------------------------
        ones16 = cpool.tile([P, 1], FP16, tag="ones16")
        nc.vector.memset(ones16[:], 1.0)
        onesf = cpool.tile([1, P], F32, tag="onesf")
        nc.vector.memset(onesf[:], 1.0)
        maskbias = cpool.tile([P, 1], FP16, tag="mb")
        nc.vector.memset(maskbias[:], 0.0)
        neg6e4 = cpool.tile([1, 1], FP16, tag="neg6e4")
        nc.vector.memset(neg6e4[:], -60000.0)
        # DMA can write partition 127 where engine APs cannot
        nc.sync.dma_start(maskbias[P - 1 : P, 0:1], neg6e4[0:1, 0:1])
        # identity for PE-transpose (tail max chain)
        ident = cpool.tile([P, P], F32, tag="ident")
        nc.gpsimd.memset(ident[:], 0.0)
        nc.gpsimd.affine_select(
            out=ident[:],
            in_=ident[:],
            compare_op=ALU.not_equal,
            fill=1.0,
            base=0,
            pattern=[[-1, P]],
            channel_multiplier=1,
        )

        # partition-0 staging: [0:256] unnormalized att, [256:288] Z partials
        stage = cpool.tile([1, BPC, 288], F32, tag="stage")
        att_n = cpool.tile([1, BPC, D], F32, tag="attn")

        # all 8 Fh broadcasts up front: PE ones-matmul + ACT cast copy
        Fh = []
        for b in range(BPC):
            ps = psf_pool.tile([P, D], F32, tag="psf", name=f"psfb{b}")
            nc.tensor.matmul(ps[:], lhsT=onesf[:], rhs=F_all[0:1, b, :], start=True, stop=True)
            fh = fpool.tile([P, D], FP16, tag=f"fh{b}", name=f"fh{b}")
            nc.scalar.activation(fh[:], ps[:], AF.Copy)
            Fh.append(fh)

        def keep_warm(anchor_ap, idx):
            # tiny matmul reading S (3-deep pool -> no DVE coupling):
            # keeps the PE HAM window from ever going fully idle near the tail
            dps = psf_pool.tile([1, 64], F32, tag="dum", name=f"dum{idx}")
            w = anchor_ap.free_size()
            nc.tensor.matmul(
                dps[0:1, 0:w], lhsT=ones16[:], rhs=anchor_ap, start=True, stop=True
            )

        def pe_max_chain(rm, tagsfx):
            # deterministic cross-partition max: rm [P,1] -> PE transpose ->
            # DVE reduce_max on partition 0 -> scalar [1,1]
            pmax = psf_pool.tile([1, P], F32, tag="pmax", name=f"pmax{tagsfx}")
            nc.tensor.transpose(pmax[:], rm[:], ident[:])
            gls = stat.tile([1, 1], F32, tag=f"gl{tagsfx}", name=f"gl{tagsfx}")
            nc.vector.reduce_max(gls[:], pmax[0:1, :], axis=AX.X)
            return gls

        def bcast_neg(gsc, tagsfx):
            # scalar [1,1] -> [P,1] negated bias for the ACT exp
            pbc = psf_pool.tile([P, 1], F32, tag="pbc", name=f"pbc{tagsfx}")
            nc.tensor.matmul(pbc[:], lhsT=onesf[:], rhs=gsc[:], start=True, stop=True)
            negk = stat.tile([P, 1], F32, tag=f"ng{tagsfx}", name=f"ng{tagsfx}")
            nc.scalar.activation(negk[:], pbc[:], AF.Copy, scale=-1.0)
            return negk

        rz = stat.tile([1, BPC], F32, tag="rz", bufs=1)

        def epilogue(b):
            # Z(b) via ACT accum_out (keeps DVE to just the reciprocal);
            # ACT then does the normalize copy
            zb = stat.tile([1, 1], F32, tag="zb", name="zb")
            zscr = stat.tile([1, NBLK], F32, tag="zscr", name="zscr")
            nc.scalar.activation(
                zscr[0:1, 0:NBLK], stage[0:1, b, D : D + NBLK], AF.Copy,
                accum_out=zb[:],
            )
            nc.vector.reciprocal(rz[0:1, b : b + 1], zb[:])
            nc.scalar.activation(
                att_n[0:1, b, :], stage[0:1, b, 0:D], AF.Copy,
                scale=rz[0:1, b : b + 1],
            )

        def mul_l1(b, lo, hi, prod, l1):
            w = hi - lo
            nc.vector.tensor_mul(
                prod[:, lo:hi, :],
                Xh[b][:, lo:hi, :],
                Fh[b][:].unsqueeze(1).broadcast_to((P, w, D)),
            )
            nc.vector.tensor_add(
                l1[:, lo:hi, :], prod[:, lo:hi, 0 : D // 2], prod[:, lo:hi, D // 2 : D]
            )

        def tree_tail(lo, hi, l1, l2, l3, l4, S):
            nc.vector.tensor_add(
                l2[:, lo:hi, :], l1[:, lo:hi, 0 : D // 4], l1[:, lo:hi, D // 4 : D // 2]
            )
            nc.vector.tensor_add(
                l3[:, lo:hi, :], l2[:, lo:hi, 0 : D // 8], l2[:, lo:hi, D // 8 : D // 4]
            )
            nc.vector.tensor_add(
                l4[:, lo:hi, :], l3[:, lo:hi, 0 : D // 16], l3[:, lo:hi, D // 16 : D // 8]
            )
            with nc.allow_low_precision(reason="scores tree is already fp16"):
                nc.vector.reduce_sum(S[:, lo:hi], l4[:, lo:hi, :], axis=AX.X)

        def mask_self(S):
            nc.vector.tensor_add(
                S[:, NBLK - 1 : NBLK], S[:, NBLK - 1 : NBLK], maskbias[:]
            )

        def pass2(Pw, bank, b, lo, hi, zw):
            for i in range(lo, hi):
                nc.tensor.matmul(
                    bank[0:1, 0:D],
                    lhsT=Pw[:, i - lo : i - lo + 1],
                    rhs=Xh[b][:, i, :],
                    start=(i == lo),
                    stop=(i == hi - 1),
                )
            nc.tensor.matmul(
                bank[0:1, D : D + zw], lhsT=ones16[:], rhs=Pw[:, 0:zw],
                start=True, stop=True,
            )

        def scr_tiles():
            prod = scrpool.tile([P, NBLK, D], FP16, tag="prod", name="prod")
            l1 = scrpool.tile([P, NBLK, D // 2], FP16, tag="l1", name="l1")
            l2 = scrpool.tile([P, NBLK, D // 4], FP16, tag="l2", name="l2")
            l3 = scrpool.tile([P, NBLK, D // 8], FP16, tag="l3", name="l3")
            l4 = scrpool.tile([P, NBLK, D // 16], FP16, tag="l4", name="l4")
            return prod, l1, l2, l3, l4

        # ------------------- samples 0..5 (steady state) -------------------
        for b in range(6):
            S = spool.tile([P, NBLK], FP16, tag="s")
            prod, l1, l2, l3, l4 = scr_tiles()

            if b == 0:
                mul_l1(b, 0, 4, prod, l1)
                mul_l1(b, 4, 8, prod, l1)
                mul_l1(b, 8, 16, prod, l1)
                mul_l1(b, 16, 32, prod, l1)
            else:
                mul_l1(b, 0, CB, prod, l1)
                mul_l1(b, CB, NBLK, prod, l1)
            tree_tail(0, NBLK, l1, l2, l3, l4, S)

            mask_self(S)
            if b == 5:
                keep_warm(S[:, 0:4], "5w")
            rm = stat.tile([P, 1], F32, tag="rm")
            nc.vector.reduce_max(rm[:], S[:], axis=AX.X)
            # cross-partition max on gpsimd (off the critical path here)
            gmax = stat.tile([P, 1], F32, tag="gm")
            nc.gpsimd.partition_all_reduce(
                gmax[:], rm[:], channels=P, reduce_op=bass_isa.ReduceOp.max
            )
            # late DMA triggers behind early all-reduces so their
            # buffer-release waits never block a needed gpsimd op
            if b == 1:
                trig(6, 0, CB)
                trig(6, CB, NBLK)
            elif b == 2:
                trig(7, 0, CB)
                trig(7, CB, 24)
                trig(7, 24, 28)
                trig(7, 28, 32)
            negmax = stat.tile([P, 1], F32, tag="nm")
            nc.scalar.activation(negmax[:], gmax[:], AF.Copy, scale=-1.0)

            Pw = spool.tile([P, NBLK], FP16, tag="pw")
            nc.scalar.activation(Pw[:], S[:], AF.Exp, bias=negmax[:], scale=1.0)

            bank = pspool.tile([1, 512], F32, tag="attps", name="attps")
            pass2(Pw, bank, b, 0, NBLK, NBLK)
            nc.scalar.activation(stage[0:1, b, 0:288], bank[0:1, 0:288], AF.Copy)
            if b > 0:
                epilogue(b - 1)

        # ------------------- sample 6: PE-chain softmax --------------------
        b = 6
        S6 = spool.tile([P, NBLK], FP16, tag="s", name="s6")
        prod6, l16, l26, l36, l46 = scr_tiles()
        mul_l1(b, 0, CB, prod6, l16)
        mul_l1(b, CB, NBLK, prod6, l16)
        tree_tail(0, NBLK, l16, l26, l36, l46, S6)
        mask_self(S6)
        keep_warm(S6[:, 0:4], "6w")
        rm6 = stat.tile([P, 1], F32, tag="rm", name="rm6")
        nc.vector.reduce_max(rm6[:], S6[:], axis=AX.X)
        g6 = pe_max_chain(rm6, "6")
        ng6 = bcast_neg(g6, "6")
        Pw6 = spool.tile([P, NBLK], FP16, tag="pw", name="pw6")
        nc.scalar.activation(Pw6[:], S6[:], AF.Exp, bias=ng6[:], scale=1.0)
        bank6 = pspool.tile([1, 512], F32, tag="attps", name="attps6")
        pass2(Pw6, bank6, b, 0, NBLK, NBLK)
        nc.scalar.activation(stage[0:1, b, 0:288], bank6[0:1, 0:288], AF.Copy)
        epilogue(5)

        # ------------------- sample 7: 3-piece flash tail ------------------
        b = 7
        S7 = spool.tile([P, NBLK], FP16, tag="s", name="s7")
        prod7, l17, l27, l37, l47 = scr_tiles()
        PIECES = [(0, 16), (16, 24), (24, 32)]
        gs = []       # cumulative max scalars [1,1]
        banks = []
        sbs = []
        gcs = None
        for k, (lo, hi) in enumerate(PIECES):
            w = hi - lo
            if k == 2:
                # split the final piece's muls 4+4 for data pipelining
                mul_l1(b, 24, 28, prod7, l17)
                mul_l1(b, 28, 32, prod7, l17)
            else:
                mul_l1(b, lo, hi, prod7, l17)
            tree_tail(lo, hi, l17, l27, l37, l47, S7)
            if hi == NBLK:
                mask_self(S7)
            keep_warm(S7[:, lo : lo + 4], f"7p{k}")
            rmk = stat.tile([P, 1], F32, tag=f"rm7{k}", name=f"rm7{k}")
            nc.vector.reduce_max(rmk[:], S7[:, lo:hi], axis=AX.X)
            gls = pe_max_chain(rmk, f"7{k}")
            if k > 0:
                gnew = stat.tile([1, 1], F32, tag=f"gc7{k}", name=f"gc7{k}")
                nc.vector.tensor_max(gnew[:], gls[:], gcs[:])
                gcs = gnew
            else:
                gcs = gls
            gs.append(gcs)
            negk = bcast_neg(gcs, f"7{k}")
            Pwk = spool.tile([P, CB], FP16, tag=f"pw7{k}", name=f"pw7{k}")
            if w < CB:
                nc.vector.memset(Pwk[:, w:CB], 0.0)
            nc.scalar.activation(
                Pwk[:, 0:w], S7[:, lo:hi], AF.Exp, bias=negk[:], scale=1.0
            )
            bk = pspool.tile([1, 512], F32, tag="attps", name=f"att7{k}")
            pass2(Pwk[:, 0:CB], bk, b, lo, hi, CB)
            banks.append(bk)
            if k < len(PIECES) - 1:
                sbk = stat.tile([1, D + CB], F32, tag=f"sb7{k}", name=f"sb7{k}")
                nc.scalar.activation(sbk[:], bk[0:1, 0 : D + CB], AF.Copy)
                sbs.append(sbk)
            if k == 0:
                # sample 6's epilogue here so att_n[6] (and the rows-0..6
                # output DMA) don't queue behind sample 7's whole tail
                epilogue(6)
        # alphas a_k = exp(gm_k - gmF); final piece used the true max
        negF = stat.tile([1, 1], F32, tag="negF", name="negF")
        nc.scalar.activation(negF[:], gcs[:], AF.Copy, scale=-1.0)
        alphas = []
        for k in range(2):
            ak = stat.tile([1, 1], F32, tag=f"al7{k}", name=f"al7{k}")
            nc.scalar.activation(
                ak[:], gs[k][:], AF.Exp, bias=negF[0:1, 0:1], scale=1.0
            )
            alphas.append(ak)
        # combine att+Z in one strip: stage7 = a0*sb0 + a1*sb1 + bank2
        u1 = stat.tile([1, D + CB], F32, tag="u1", name="u1")
        nc.vector.scalar_tensor_tensor(
            out=u1[:], in0=sbs[1][:], scalar=alphas[1][0:1, 0:1],
            in1=banks[2][0:1, 0 : D + CB], op0=ALU.mult, op1=ALU.add,
        )
        nc.vector.scalar_tensor_tensor(
            out=stage[0:1, b, 0 : D + CB], in0=sbs[0][:],
            scalar=alphas[0][0:1, 0:1],
            in1=u1[:], op0=ALU.mult, op1=ALU.add,
        )
        # Z, reciprocal, normalize
        zb7 = stat.tile([1, 1], F32, tag="zb7", name="zb7")
        zscr7 = stat.tile([1, CB], F32, tag="z7scr", name="z7scr")
        nc.scalar.activation(
            zscr7[:], stage[0:1, b, D : D + CB], AF.Copy, accum_out=zb7[:]
        )
        nc.vector.reciprocal(rz[0:1, b : b + 1], zb7[:])
        nc.scalar.activation(
            att_n[0:1, b, :], stage[0:1, b, 0:D], AF.Copy,
            scale=rz[0:1, b : b + 1],
        )

        # rows 0..6 can fly as soon as their normalize copies land;
        # only sample 7's 1KB row remains on the tail
        nc.sync.dma_start(
            oap[0 : BPC - 1, D : 2 * D].unsqueeze(0), att_n[0:1, 0 : BPC - 1, :]
        )
        nc.sync.dma_start(
            oap[BPC - 1 : BPC, D : 2 * D].unsqueeze(0), att_n[0:1, BPC - 1, :].unsqueeze(1)
        )

    nc.compile()
    return nc


def _run(x, trace=False):
    global _NC_CACHE
    x = np.ascontiguousarray(np.asarray(x, dtype=np.float32))
    assert x.shape == (B, T, D), x.shape
    if _NC_CACHE is None:
        _NC_CACHE = _build()
    in_maps = [{"x": x[c * BPC : (c + 1) * BPC]} for c in range(N_CORES)]
    res = run_bass_kernel_spmd(
        _NC_CACHE, in_maps, core_ids=list(range(N_CORES)), trace=trace
    )
    out = np.concatenate([res.results[c]["out"] for c in range(N_CORES)], axis=0)
    return out.astype(np.float32), res


def kernel(x):
    out, _ = _run(x, trace=False)
    return out


# revision 26
# speedup vs baseline: 1.7081x; 1.0157x over previous
"""Last-query sparse attention on 8 TRN2 NeuronCores.

Reference computation (per sample b):
    prev  = x[b, :-1, :]                 # [T-1, D]
    final = x[b, -1, :]                  # [D]
    s     = prev @ final                 # [T-1]
    w     = softmax(s)
    att   = w @ prev                     # [D]
    out   = concat(final, att)           # [2D]

Sharding: batch (B=64) split 8 ways -> 8 samples per core, no collectives.

v7 design (trace-driven):
- DMA: 16-block (2MB) SWDGE cast chunks (8KB write packets run the SDMA
  engines at their ~26GB/s limit; bigger packets measure ~20% slower).
  Samples 0-5 front-loaded (xbpool bufs=6); sample 0 ramps 4/4/8/16.
  Sample 7 loads as 16/8/4/4 pieces so the last-arriving data needs
  minimal work.
- Pass 1 on DVE (the saturated engine, ~82us): chunk-split fp16 mul+l1,
  merged l2/l3/l4 + 1x segmented reduce to fp16 S. gpsimd tensor ops
  measured 2.4x WORSE overall -- do not offload tree levels there.
- All 8 Fh broadcasts built up front (ACT FIFO otherwise parks them
  behind matmul-dependent stage copies).
- Softmax samples 0-5: DVE row-max -> gpsimd partition_all_reduce ->
  ACT negate -> ACT exp (AR is off the critical path there). Samples
  6/7 use a deterministic PE-transpose max chain (rm -> PE transpose ->
  DVE reduce_max -> PE ones-broadcast -> ACT negate): tail ARs measured
  1-5.6us and their variance poisons the in-order PE/ACT queues.
- PE keep-warm: tiny dummy matmuls anchored on S of samples 5-7 only
  (S has a 3-deep pool so the PE reader can't stall future DVE work;
  anchoring on 2-deep scr tiles serialized the whole pipeline).
- Pass 2: 32 PE matmuls/sample into a [1,512] PSUM row + ones-matmul Z.
- Sample 7: 3 processing pieces (16/8/8; the last piece's muls split
  4+4 for data pipelining) with cumulative max, per-piece banks with
  zero-padded 16-wide Z, single end combine (2 STT adds over [0:272]),
  Z via one ACT accum_out.
"""

import sys

sys.path.insert(0, "/opt/trn_rl_repo")

from contextlib import ExitStack

import numpy as np

import concourse.tile as tile
import concourse.bass_isa as bass_isa
from concourse import bacc, mybir
from concourse.bass_utils import run_bass_kernel_spmd

N_CORES = 8
B = 64
T = 4096
D = 256
BPC = B // N_CORES  # samples per core
P = 128
NBLK = T // P  # 32 blocks; t = p*NBLK + i
CB = 16  # blocks per chunk
F32 = mybir.dt.float32
FP16 = mybir.dt.float16
AX = mybir.AxisListType
ALU = mybir.AluOpType

_NC_CACHE = None


def _build():
    AF = mybir.ActivationFunctionType
    nc = bacc.Bacc(
        trn_type="TRN2",
        target_bir_lowering=False,
        debug=False,
        num_devices=N_CORES,
    )
    x_ext = nc.declare_dram_parameter("x", [BPC, T, D], F32, isOutput=False)
    out_ext = nc.declare_dram_parameter("out", [BPC, 2 * D], F32, isOutput=True)
    xap = x_ext.ap()
    oap = out_ext.ap()

    with ExitStack() as ctx:
        tc = ctx.enter_context(tile.TileContext(nc))
        xbpool = ctx.enter_context(tc.tile_pool(name="xbp", bufs=6))
        fpool = ctx.enter_context(tc.tile_pool(name="fp", bufs=1))
        scrpool = ctx.enter_context(tc.tile_pool(name="scr", bufs=2))
        spool = ctx.enter_context(tc.tile_pool(name="sp", bufs=3))
        stat = ctx.enter_context(tc.tile_pool(name="stat", bufs=3))
        cpool = ctx.enter_context(tc.tile_pool(name="const", bufs=1))
        pspool = ctx.enter_context(tc.tile_pool(name="ps", bufs=4, space="PSUM"))
        psf_pool = ctx.enter_context(tc.tile_pool(name="psf", bufs=1, space="PSUM"))

        xr = [xap[b].rearrange("(p i) d -> p i d", p=P) for b in range(BPC)]

        # --- earliest DMAs ------------------------------------------------
        # final-row gather on the HWDGE/sync path first: it feeds fh_bcast
        F_all = cpool.tile([1, BPC, D], F32, tag="fall")
        nc.sync.dma_start(F_all[0:1, :, :], xap[:, T - 1, :].unsqueeze(0))

        Xh = [xbpool.tile([P, NBLK, D], FP16, tag="xh", name="xh") for _ in range(BPC)]

        def trig(b, lo, hi):
            nc.gpsimd.dma_start(Xh[b][:, lo:hi, :], xr[b][:, lo:hi, :])

        # sample 0 ramps 4/4/8/16; 1-5 as 16-block chunks
        trig(0, 0, 4)
        trig(0, 4, 8)
        trig(0, 8, 16)
        trig(0, 16, 32)
        for b in range(1, 6):
            trig(b, 0, CB)
            trig(b, CB, NBLK)

        # final half of the output: straight HBM->HBM copy (sync engine)
        nc.sync.dma_start(oap[:, 0:D], xap[:, T - 1, :])

        # --- constants ----------------------------------------------------
        ones16 = cpool.tile([P, 1], FP16, tag="ones16")
        nc.vector.memset(ones16[:], 1.0)
        onesf = cpool.tile([1, P], F32, tag="onesf")
        nc.vector.memset(onesf[:], 1.0)
        maskbias = cpool.tile([P, 1], FP16, tag="mb")
        nc.vector.memset(maskbias[:], 0.0)
        neg6e4 = cpool.tile([1, 1], FP16, tag="neg6e4")
        nc.vector.memset(neg6e4[:], -60000.0)
        # DMA can write partition 127 where engine APs cannot
        nc.sync.dma_start(maskbias[P - 1 : P, 0:1], neg6e4[0:1, 0:1])
        # identity for PE-transpose (tail max chain)
        ident = cpool.tile([P, P], F32, tag="ident")
        nc.gpsimd.memset(ident[:], 0.0)
        nc.gpsimd.affine_select(
            out=ident[:],
            in_=ident[:],
            compare_op=ALU.not_equal,
            fill=1.0,
            base=0,
            pattern=[[-1, P]],
            channel_multiplier=1,
        )

        # partition-0 staging: [0:256] unnormalized att, [256:288] Z partials
        stage = cpool.tile([1, BPC, 288], F32, tag="stage")
        att_n = cpool.tile([1, BPC, D], F32, tag="attn")

        # all 8 Fh broadcasts up front: PE ones-matmul + ACT cast copy
        Fh = []
        for b in range(BPC):
            ps = psf_pool.tile([P, D], F32, tag="psf", name=f"psfb{b}")
            nc.tensor.matmul(ps[:], lhsT=onesf[:], rhs=F_all[0:1, b, :], start=True, stop=True)
            fh = fpool.tile([P, D], FP16, tag=f"fh{b}", name=f"fh{b}")
            nc.scalar.activation(fh[:], ps[:], AF.Copy)
            Fh.append(fh)

        def keep_warm(anchor_ap, idx):
            # tiny matmul reading S (3-deep pool -> no DVE coupling):
            # keeps the PE HAM window from ever going fully idle near the tail
            dps = psf_pool.tile([1, 64], F32, tag="dum", name=f"dum{idx}")
            w = anchor_ap.free_size()
            nc.tensor.matmul(
                dps[0:1, 0:w], lhsT=ones16[:], rhs=anchor_ap, start=True, stop=True
            )

        def pe_max_chain(rm, tagsfx):
            # deterministic cross-partition max: rm [P,1] -> PE transpose ->
            # DVE reduce_max on partition 0 -> scalar [1,1]
            pmax = psf_pool.tile([1, P], F32, tag="pmax", name=f"pmax{tagsfx}")
            nc.tensor.transpose(pmax[:], rm[:], ident[:])
            gls = stat.tile([1, 1], F32, tag=f"gl{tagsfx}", name=f"gl{tagsfx}")
            nc.vector.reduce_max(gls[:], pmax[0:1, :], axis=AX.X)
            return gls

        def bcast_neg(gsc, tagsfx):
            # scalar [1,1] -> [P,1] negated bias for the ACT exp
            pbc = psf_pool.tile([P, 1], F32, tag="pbc", name=f"pbc{tagsfx}")
            nc.tensor.matmul(pbc[:], lhsT=onesf[:], rhs=gsc[:], start=True, stop=True)
            negk = stat.tile([P, 1], F32, tag=f"ng{tagsfx}", name=f"ng{tagsfx}")
            nc.scalar.activation(negk[:], pbc[:], AF.Copy, scale=-1.0)
            return negk

        rz = stat.tile([1, BPC], F32, tag="rz", bufs=1)

        def epilogue(b):
            # Z(b) via ACT accum_out (keeps DVE to just the reciprocal);
            # ACT then does the normalize copy
            zb = stat.tile([1, 1], F32, tag="zb", name="zb")
            zscr = stat.tile([1, NBLK], F32, tag="zscr", name="zscr")
            nc.scalar.activation(
                zscr[0:1, 0:NBLK], stage[0:1, b, D : D + NBLK], AF.Copy,
                accum_out=zb[:],
            )
            nc.vector.reciprocal(rz[0:1, b : b + 1], zb[:])
            nc.scalar.activation(
                att_n[0:1, b, :], stage[0:1, b, 0:D], AF.Copy,
                scale=rz[0:1, b : b + 1],
            )

        def mul_l1(b, lo, hi, prod, l1):
            w = hi - lo
            nc.vector.tensor_mul(
                prod[:, lo:hi, :],
                Xh[b][:, lo:hi, :],
                Fh[b][:].unsqueeze(1).broadcast_to((P, w, D)),
            )
            nc.vector.tensor_add(
                l1[:, lo:hi, :], prod[:, lo:hi, 0 : D // 2], prod[:, lo:hi, D // 2 : D]
            )

        def tree_tail(lo, hi, l1, l2, l3, l4, S):
            nc.vector.tensor_add(
                l2[:, lo:hi, :], l1[:, lo:hi, 0 : D // 4], l1[:, lo:hi, D // 4 : D // 2]
            )
            nc.vector.tensor_add(
                l3[:, lo:hi, :], l2[:, lo:hi, 0 : D // 8], l2[:, lo:hi, D // 8 : D // 4]
            )
            nc.vector.tensor_add(
                l4[:, lo:hi, :], l3[:, lo:hi, 0 : D // 16], l3[:, lo:hi, D // 16 : D // 8]
            )
            with nc.allow_low_precision(reason="scores tree is already fp16"):
                nc.vector.reduce_sum(S[:, lo:hi], l4[:, lo:hi, :], axis=AX.X)

        def mask_self(S):
            nc.vector.tensor_add(
                S[:, NBLK - 1 : NBLK], S[:, NBLK - 1 : NBLK], maskbias[:]
            )

        def pass2(Pw, bank, b, lo, hi, zw):
            for i in range(lo, hi):
                nc.tensor.matmul(
                    bank[0:1, 0:D],
                    lhsT=Pw[:, i - lo : i - lo + 1],
                    rhs=Xh[b][:, i, :],
                    start=(i == lo),
                    stop=(i == hi - 1),
                )
            nc.tensor.matmul(
                bank[0:1, D : D + zw], lhsT=ones16[:], rhs=Pw[:, 0:zw],
                start=True, stop=True,
            )

        def scr_tiles():
            prod = scrpool.tile([P, NBLK, D], FP16, tag="prod", name="prod")
            l1 = scrpool.tile([P, NBLK, D // 2], FP16, tag="l1", name="l1")
            l2 = scrpool.tile([P, NBLK, D // 4], FP16, tag="l2", name="l2")
            l3 = scrpool.tile([P, NBLK, D // 8], FP16, tag="l3", name="l3")
            l4 = scrpool.tile([P, NBLK, D // 16], FP16, tag="l4", name="l4")
            return prod, l1, l2, l3, l4

        # ------------------- samples 0..5 (steady state) -------------------
        for b in range(6):
            S = spool.tile([P, NBLK], FP16, tag="s")
            prod, l1, l2, l3, l4 = scr_tiles()

            if b == 0:
                mul_l1(b, 0, 4, prod, l1)
                mul_l1(b, 4, 8, prod, l1)
                mul_l1(b, 8, 16, prod, l1)
                mul_l1(b, 16, 32, prod, l1)
            else:
                mul_l1(b, 0, CB, prod, l1)
                mul_l1(b, CB, NBLK, prod, l1)
            tree_tail(0, NBLK, l1, l2, l3, l4, S)

            mask_self(S)
            if b == 5:
                keep_warm(S[:, 0:4], "5w")
            rm = stat.tile([P, 1], F32, tag="rm")
            nc.vector.reduce_max(rm[:], S[:], axis=AX.X)
            # cross-partition max on gpsimd (off the critical path here)
            gmax = stat.tile([P, 1], F32, tag="gm")
            nc.gpsimd.partition_all_reduce(
                gmax[:], rm[:], channels=P, reduce_op=bass_isa.ReduceOp.max
            )
            # late DMA triggers behind early all-reduces so their
            # buffer-release waits never block a needed gpsimd op
            if b == 1:
                trig(6, 0, CB)
                trig(6, CB, NBLK)
            elif b == 2:
                trig(7, 0, CB)
                trig(7, CB, 24)
                trig(7, 24, 28)
                trig(7, 28, 32)
            negmax = stat.tile([P, 1], F32, tag="nm")
            nc.scalar.activation(negmax[:], gmax[:], AF.Copy, scale=-1.0)

            Pw = spool.tile([P, NBLK], FP16, tag="pw")
            nc.scalar.activation(Pw[:], S[:], AF.Exp, bias=negmax[:], scale=1.0)

            bank = pspool.tile([1, 512], F32, tag="attps", name="attps")
            pass2(Pw, bank, b, 0, NBLK, NBLK)
            nc.scalar.activation(stage[0:1, b, 0:288], bank[0:1, 0:288], AF.Copy)
            # epilogue lags TWO samples: its DVE reciprocal must never pop
            # while its ACT-accumulated Z is still queued (in-order DVE stall)
            if b > 1:
                epilogue(b - 2)

        # ------------------- sample 6: PE-chain softmax --------------------
        b = 6
        S6 = spool.tile([P, NBLK], FP16, tag="s", name="s6")
        prod6, l16, l26, l36, l46 = scr_tiles()
        mul_l1(b, 0, CB, prod6, l16)
        mul_l1(b, CB, NBLK, prod6, l16)
        tree_tail(0, NBLK, l16, l26, l36, l46, S6)
        mask_self(S6)
        keep_warm(S6[:, 0:4], "6w")
        rm6 = stat.tile([P, 1], F32, tag="rm", name="rm6")
        nc.vector.reduce_max(rm6[:], S6[:], axis=AX.X)
        g6 = pe_max_chain(rm6, "6")
        ng6 = bcast_neg(g6, "6")
        Pw6 = spool.tile([P, NBLK], FP16, tag="pw", name="pw6")
        nc.scalar.activation(Pw6[:], S6[:], AF.Exp, bias=ng6[:], scale=1.0)
        bank6 = pspool.tile([1, 512], F32, tag="attps", name="attps6")
        pass2(Pw6, bank6, b, 0, NBLK, NBLK)
        nc.scalar.activation(stage[0:1, b, 0:288], bank6[0:1, 0:288], AF.Copy)
        epilogue(4)

        # ------------------- sample 7: 3-piece flash tail ------------------
        b = 7
        S7 = spool.tile([P, NBLK], FP16, tag="s", name="s7")
        prod7, l17, l27, l37, l47 = scr_tiles()
        PIECES = [(0, 16), (16, 24), (24, 32)]
        gs = []       # cumulative max scalars [1,1]
        banks = []
        sbs = []
        gcs = None
        for k, (lo, hi) in enumerate(PIECES):
            w = hi - lo
            if k == 2:
                # split the final piece's muls 4+4 for data pipelining
                mul_l1(b, 24, 28, prod7, l17)
                mul_l1(b, 28, 32, prod7, l17)
            else:
                mul_l1(b, lo, hi, prod7, l17)
            tree_tail(lo, hi, l17, l27, l37, l47, S7)
            if hi == NBLK:
                mask_self(S7)
            keep_warm(S7[:, lo : lo + 4], f"7p{k}")
            rmk = stat.tile([P, 1], F32, tag=f"rm7{k}", name=f"rm7{k}")
            nc.vector.reduce_max(rmk[:], S7[:, lo:hi], axis=AX.X)
            gls = pe_max_chain(rmk, f"7{k}")
            if k > 0:
                gnew = stat.tile([1, 1], F32, tag=f"gc7{k}", name=f"gc7{k}")
                nc.vector.tensor_max(gnew[:], gls[:], gcs[:])
                gcs = gnew
            else:
                gcs = gls
            gs.append(gcs)
            negk = bcast_neg(gcs, f"7{k}")
            Pwk = spool.tile([P, CB], FP16, tag=f"pw7{k}", name=f"pw7{k}")
            if w < CB:
                nc.vector.memset(Pwk[:, w:CB], 0.0)
            nc.scalar.activation(
                Pwk[:, 0:w], S7[:, lo:hi], AF.Exp, bias=negk[:], scale=1.0
            )
            bk = pspool.tile([1, 512], F32, tag="attps", name=f"att7{k}")
            pass2(Pwk[:, 0:CB], bk, b, lo, hi, CB)
            banks.append(bk)
            if k < len(PIECES) - 1:
                sbk = stat.tile([1, D + CB], F32, tag=f"sb7{k}", name=f"sb7{k}")
                nc.scalar.activation(sbk[:], bk[0:1, 0 : D + CB], AF.Copy)
                sbs.append(sbk)
            if k == 0:
                # sample 5's epilogue here (Z long since accumulated)
                epilogue(5)
        # alphas a_k = exp(gm_k - gmF); final piece used the true max
        negF = stat.tile([1, 1], F32, tag="negF", name="negF")
        nc.scalar.activation(negF[:], gcs[:], AF.Copy, scale=-1.0)
        alphas = []
        for k in range(2):
            ak = stat.tile([1, 1], F32, tag=f"al7{k}", name=f"al7{k}")
            nc.scalar.activation(
                ak[:], gs[k][:], AF.Exp, bias=negF[0:1, 0:1], scale=1.0
            )
            alphas.append(ak)
        # sample 6 epilogue after the alphas (ACT) and before the combine
        # STTs (DVE): both its inputs are ready so neither queue stalls
        epilogue(6)
        # combine att+Z in one strip: stage7 = a0*sb0 + a1*sb1 + bank2
        u1 = stat.tile([1, D + CB], F32, tag="u1", name="u1")
        nc.vector.scalar_tensor_tensor(
            out=u1[:], in0=sbs[1][:], scalar=alphas[1][0:1, 0:1],
            in1=banks[2][0:1, 0 : D + CB], op0=ALU.mult, op1=ALU.add,
        )
        nc.vector.scalar_tensor_tensor(
            out=stage[0:1, b, 0 : D + CB], in0=sbs[0][:],
            scalar=alphas[0][0:1, 0:1],
            in1=u1[:], op0=ALU.mult, op1=ALU.add,
        )
        # Z, reciprocal, normalize
        zb7 = stat.tile([1, 1], F32, tag="zb7", name="zb7")
        zscr7 = stat.tile([1, CB], F32, tag="z7scr", name="z7scr")
        nc.scalar.activation(
            zscr7[:], stage[0:1, b, D : D + CB], AF.Copy, accum_out=zb7[:]
        )
        nc.vector.reciprocal(rz[0:1, b : b + 1], zb7[:])
        nc.scalar.activation(
            att_n[0:1, b, :], stage[0:1, b, 0:D], AF.Copy,
            scale=rz[0:1, b : b + 1],
        )

        # rows 0..6 can fly as soon as their normalize copies land;
        # only sample 7's 1KB row remains on the tail
        nc.sync.dma_start(
            oap[0 : BPC - 1, D : 2 * D].unsqueeze(0), att_n[0:1, 0 : BPC - 1, :]
        )
        nc.sync.dma_start(
            oap[BPC - 1 : BPC, D : 2 * D].unsqueeze(0), att_n[0:1, BPC - 1, :].unsqueeze(1)
        )

    nc.compile()
    return nc


def _run(x, trace=False):
    global _NC_CACHE
    x = np.ascontiguousarray(np.asarray(x, dtype=np.float32))
    assert x.shape == (B, T, D), x.shape
    if _NC_CACHE is None:
        _NC_CACHE = _build()
    in_maps = [{"x": x[c * BPC : (c + 1) * BPC]} for c in range(N_CORES)]
    res = run_bass_kernel_spmd(
        _NC_CACHE, in_maps, core_ids=list(range(N_CORES)), trace=trace
    )
    out = np.concatenate([res.results[c]["out"] for c in range(N_CORES)], axis=0)
    return out.astype(np.float32), res


def kernel(x):
    out, _ = _run(x, trace=False)
    return out
